# revision 47
# baseline (speedup 1.0000x reference)
"""BiAttention kernel for Trainium2 (Bass/Tile), data-parallel over batch on 8 cores.

Math (per batch b):
  att[l,m] = s_in[l] + g[m] + S[l,m]
    S[l,m]  = sum_d inp[l,d]*dot_scale[d]*mem[m,d]
    s_in[l] = sum_d inp[l,d]*w_input[d]
    g[m]    = sum_d mem[m,d]*w_memory[d] + (mask[m]-1)*1e30
  weight_one = softmax_m(att) = softmax_m(S + g)           (s_in cancels)
  output_one = weight_one @ mem
  w2u[l] = exp(max_m att[l,:]) = max_m exp(S+g) * exp(s_in[l])
  output_two = (w2u/sum w2u) @ inp
  out = concat([inp, output_one, inp*output_one, output_two*output_one], -1)

Implementation: S computed transposed (S_T[m,l]) so that P~ = exp(S_T + g) is
directly the lhsT of the second matmul.  Softmax denominators come for free
from a ones-column appended to mem in the second matmul.  max_m att recovered
from max_m P~ (exp is monotone), so no log is needed:
w2u = maxP~ * exp(s_in).
"""

import threading

import numpy as np

import concourse.bacc as bacc
import concourse.bass as bass
import concourse.mybir as mybir
import concourse.tile as tile
from concourse.masks import make_identity

F32 = mybir.dt.float32
BF16 = mybir.dt.bfloat16
AF = mybir.ActivationFunctionType
ALU = mybir.AluOpType
AX = mybir.AxisListType

B, L, M, D = 8, 2048, 2048, 256
P = 128
LT = L // P          # 16 l-tiles
MT = M // P          # 16 m-tiles
KD = D // P          # 2 contraction tiles
NQ = 4               # l-quarters (PSUM-accumulator constraint)
QW = L // NQ         # 512
QT = QW // P         # 4 l-tiles per quarter
NEG_BIG = 1.0e30


def build_nc():
    nc = bacc.Bacc(
        "TRN2", target_bir_lowering=False, debug=False, num_devices=8
    )

    inp_d = nc.dram_tensor("input", [L, D], F32, kind="ExternalInput").ap()
    mem_d = nc.dram_tensor("memory", [M, D], F32, kind="ExternalInput").ap()
    mask_d = nc.dram_tensor("mask", [M], F32, kind="ExternalInput").ap()
    w_in_d = nc.dram_tensor("w_input", [D], F32, kind="ExternalInput").ap()
    w_mem_d = nc.dram_tensor("w_memory", [D], F32, kind="ExternalInput").ap()
    dsc_d = nc.dram_tensor("dot_scale", [D], F32, kind="ExternalInput").ap()
    out_d = nc.dram_tensor("out", [L, 4 * D], F32, kind="ExternalOutput").ap()

    inp_r = inp_d.rearrange("(t p) d -> p t d", p=P)      # [128,16,256]
    mem_r = mem_d.rearrange("(t p) d -> p t d", p=P)      # [128,16,256]
    mask_r = mask_d.rearrange("(t p) -> t p", p=P)        # [16,128]
    out_r = out_d.rearrange("(t p) c -> p t c", p=P)      # [128,16,1024]

    with tile.TileContext(nc) as tc:
        with (
            tc.tile_pool(name="consts", bufs=1) as cp,
            tc.tile_pool(name="ptiles", bufs=3) as pp,
            tc.tile_pool(name="stage", bufs=4) as sp,
            tc.tile_pool(name="dots", bufs=2) as dp,
            tc.tile_pool(name="rp", bufs=4) as rp,
            tc.tile_pool(name="psS", bufs=2, space="PSUM") as psS,
            tc.tile_pool(name="psM", bufs=2, space="PSUM") as psM,
            tc.tile_pool(name="psAcc", bufs=4, space="PSUM") as psA,
        ):
            # ---------------- persistent SBUF ----------------
            ident_f = cp.tile([P, P], F32)
            ident_b = cp.tile([P, P], BF16)
            make_identity(nc, ident_f)
            make_identity(nc, ident_b)

            in_sb = cp.tile([P, LT, D], F32)       # natural input
            in_bf = cp.tile([P, LT, D], BF16)      # bf16 copy for transposing
            mem_sb = cp.tile([P, MT, D + 1], BF16)  # natural memory + ones col
            inT = cp.tile([P, KD, L], BF16)        # scaled input^T  [d, l]
            memT = cp.tile([P, KD, M], BF16)       # memory^T [d, m]
            maxacc = cp.tile([P, L], BF16)         # running max of P~ over m-tiles
            out1_sb = cp.tile([P, LT, D], F32)     # normalized output_one
            mask_pad = cp.tile([P, P], F32)        # mask rows 0:16, rest garbage
            vpad = cp.tile([P, P], F32)            # dot_scale rows 0:2, w_memory rows 2:4
            w_in_row = cp.tile([1, D], F32)
            w_in_bc = cp.tile([P, D], F32)
            w_mem_sb = cp.tile([P, KD], BF16)
            dsc_sb = cp.tile([P, KD], F32)
            g_sb = cp.tile([P, MT], F32)
            gtmp_sb = cp.tile([P, MT], F32)
            s_in_sb = cp.tile([P, LT], F32)
            exp_si = cp.tile([P, LT], F32)
            rowmax = cp.tile([P, LT], F32)
            w2u = cp.tile([P, LT], F32)
            w2s = cp.tile([P, 1], F32)
            ones_col = cp.tile([P, 1], F32)
            ones_row = cp.tile([1, P], F32)
            rtot = cp.tile([1, 1], F32)
            rtot_b = cp.tile([P, 1], F32)

            # ---------------- loads (small params first) ----------------
            nc.sync.dma_start(out=vpad[0:KD, :], in_=dsc_d.rearrange("(k p) -> k p", p=P))
            nc.sync.dma_start(
                out=vpad[KD : 2 * KD, :], in_=w_mem_d.rearrange("(k p) -> k p", p=P)
            )
            nc.sync.dma_start(out=w_in_row[:], in_=w_in_d[None, :])
            nc.sync.dma_start(out=mask_pad[0:MT, :], in_=mask_r)
            nc.gpsimd.memset(ones_col[:], 1.0)
            nc.gpsimd.memset(ones_row[:], 1.0)
            # touch Exp early so the ACT table load happens off the critical path
            warm = cp.tile([P, 1], F32)
            nc.scalar.activation(out=warm[:], in_=ones_col[:], func=AF.Exp)
            for c in range(8):  # 2-tile chunks matching transpose batches
                nc.sync.dma_start(
                    out=in_sb[:, c * 2 : (c + 1) * 2, :],
                    in_=inp_r[:, c * 2 : (c + 1) * 2, :],
                )
                nc.gpsimd.dma_start(
                    out=in_bf[:, c * 2 : (c + 1) * 2, :],
                    in_=inp_r[:, c * 2 : (c + 1) * 2, :],
                )  # f32 -> bf16 cast
                nc.gpsimd.dma_start(
                    out=mem_sb[:, c * 2 : (c + 1) * 2, 0:D],
                    in_=mem_r[:, c * 2 : (c + 1) * 2, :],
                )  # f32 -> bf16 cast
            nc.gpsimd.memset(mem_sb[:, :, D : D + 1], 1.0)

            # ---------------- small params ----------------
            pv = psS.tile([P, P], F32, tag="s")
            nc.tensor.transpose(pv[:], vpad[:], ident_f[:])
            nc.vector.tensor_copy(dsc_sb[:], pv[:, 0:KD])
            nc.vector.tensor_copy(w_mem_sb[:], pv[:, KD : 2 * KD])  # cast to bf16
            # broadcast w_input across partitions via ones-matmul
            wb = psS.tile([P, D], F32, tag="s")
            nc.tensor.matmul(wb[:], lhsT=ones_row[:], rhs=w_in_row[:], start=True, stop=True)
            nc.vector.tensor_copy(w_in_bc[:], wb[:])

            # mask term: (mask-1)*1e30 via padded full-K transpose
            mtp = psS.tile([P, P], F32, tag="s")  # mask transposed (cols 0:16 valid)
            nc.tensor.transpose(mtp[:], mask_pad[:], ident_f[:])
            nc.vector.tensor_scalar(
                out=gtmp_sb[:], in0=mtp[:, 0:MT], scalar1=1.0, scalar2=NEG_BIG,
                op0=ALU.subtract, op1=ALU.mult,
            )

            # ---------------- big transposes ----------------
            # batch 4 [128,128] transposes per psum tile; batch order (k, t)
            # matches dest AP [p, k, t*128:(t+2)*128].
            # Order: input batches 0-1 (cover all of q=0), then memory batches
            # (gate g and mm1 lhsT), then remaining input batches.
            mdp = psA.tile([P, MT], F32, tag="acc")
            slot = [0]

            def input_batch(bi):
                t0 = 2 * bi
                pool = psA if slot[0] % 2 else psS
                slot[0] += 1
                ptr = pool.tile(
                    [P, 512], BF16, name=f"ptri{bi}",
                    tag="acc" if pool is psA else "s",
                )
                j = 0
                for k in range(KD):
                    for t in (t0, t0 + 1):
                        nc.tensor.transpose(
                            ptr[:, j * P : (j + 1) * P],
                            in_bf[:, t, k * P : (k + 1) * P],
                            ident_b,
                        )
                        j += 1
                for k in range(KD):
                    if bi < 2:
                        # ACT is idle before the exp stream starts
                        nc.scalar.activation(
                            out=inT[:, k, t0 * P : (t0 + 2) * P],
                            in_=ptr[:, k * 2 * P : (k + 1) * 2 * P],
                            func=AF.Copy, scale=dsc_sb[:, k : k + 1],
                        )
                    else:
                        # keep ACT's in-order queue free for main-loop exps
                        nc.vector.tensor_scalar(
                            out=inT[:, k, t0 * P : (t0 + 2) * P],
                            in0=ptr[:, k * 2 * P : (k + 1) * 2 * P],
                            scalar1=dsc_sb[:, k : k + 1], scalar2=None,
                            op0=ALU.mult,
                        )

            def memory_batch(bi):
                t0 = 2 * bi
                pool = psA if slot[0] % 2 else psS
                slot[0] += 1
                ptr = pool.tile(
                    [P, 512], BF16, name=f"ptrm{bi}",
                    tag="acc" if pool is psA else "s",
                )
                j = 0
                for k in range(KD):
                    for t in (t0, t0 + 1):
                        nc.tensor.transpose(
                            ptr[:, j * P : (j + 1) * P],
                            mem_sb[:, t, k * P : (k + 1) * P],
                            ident_b,
                        )
                        j += 1
                nc.vector.tensor_copy(
                    out=memT[:, :, t0 * P : (t0 + 2) * P], in_=ptr[:]
                )
                # memory_dot for these two m-tiles, then the g chunk
                for t in (t0, t0 + 1):
                    for k in range(KD):
                        nc.tensor.matmul(
                            mdp[:, t : t + 1],
                            lhsT=memT[:, k, t * P : (t + 1) * P],
                            rhs=w_mem_sb[:, k : k + 1],
                            start=(k == 0),
                            stop=(k == KD - 1),
                        )
                nc.vector.tensor_add(
                    g_sb[:, t0 : t0 + 2],
                    gtmp_sb[:, t0 : t0 + 2],
                    mdp[:, t0 : t0 + 2],
                )

            input_batch(0)
            input_batch(1)
            for bi in range(8):
                memory_batch(bi)
            for bi in range(2, 8):
                input_batch(bi)

            # ---------------- main loop ----------------
            # mm1 runs one iteration ahead of exp/mm2 so the PE never idles
            # waiting on the ACT exp; o2p accumulates output_two per quarter
            # (each quarter's 4 matmuls are emitted mid-next-quarter).
            o2p = psS.tile([1, D], F32, tag="s")

            def emit_mm1(q, t, ps):
                for k in range(KD):
                    nc.tensor.matmul(
                        ps[:],
                        lhsT=memT[:, k, t * P : (t + 1) * P],
                        rhs=inT[:, k, q * QW : (q + 1) * QW],
                        start=(k == 0),
                        stop=(k == KD - 1),
                    )

            def emit_out2(qd):
                for lt in range(QT):
                    tg = qd * QT + lt
                    nc.tensor.matmul(
                        o2p[:],
                        lhsT=w2u[:, tg : tg + 1],
                        rhs=in_sb[:, tg, :],
                        start=(tg == 0),
                        stop=(tg == LT - 1),
                    )

            ps_next = psM.tile([P, QW], F32, tag="m", name="ps_q0_t0")
            emit_mm1(0, 0, ps_next)
            for q in range(NQ):
                accs = [
                    psA.tile([P, D + 1], F32, tag="acc", name=f"acc_q{q}_{i}")
                    for i in range(QT)
                ]
                for t in range(MT):
                    ps = ps_next
                    nt = q * MT + t + 1
                    if nt < NQ * MT:
                        ps_next = psM.tile(
                            [P, QW], F32, tag="m", name=f"ps_{nt}"
                        )
                        emit_mm1(nt // MT, nt % MT, ps_next)
                    pt = pp.tile([P, QW], BF16)
                    nc.scalar.activation(
                        out=pt[:], in_=ps[:], func=AF.Exp, bias=g_sb[:, t : t + 1]
                    )
                    msl = maxacc[:, q * QW : (q + 1) * QW]
                    if t == 0:
                        nc.vector.tensor_copy(msl, pt[:])
                    else:
                        nc.vector.tensor_max(msl, msl, pt[:])
                    # s_in dot-products for this quarter's l-tiles
                    if t % 4 == 1:
                        td = q * QT + t // 4
                        dump = dp.tile([P, D], F32, name=f"dmp_i{td}", tag="dump")
                        nc.vector.tensor_mul(dump[:], in_sb[:, td, :], w_in_bc[:])
                        nc.vector.reduce_sum(
                            s_in_sb[:, td : td + 1], dump[:], axis=AX.X
                        )
                    # previous quarter's output_two matmuls, mid-quarter
                    if t == 6 and q > 0:
                        emit_out2(q - 1)
                    for lt in range(QT):
                        nc.tensor.matmul(
                            accs[lt][:],
                            lhsT=pt[:, lt * P : (lt + 1) * P],
                            rhs=mem_sb[:, t, :],
                            start=(t == 0),
                            stop=(t == MT - 1),
                        )

                # row max over partitions via PE transpose + free-dim reduce
                trp = psS.tile([P, QW], BF16, tag="s")
                for lt in range(QT):
                    nc.tensor.transpose(
                        trp[:, lt * P : (lt + 1) * P],
                        maxacc[:, (q * QT + lt) * P : (q * QT + lt + 1) * P],
                        ident_b,
                    )
                nc.vector.reduce_max(
                    rowmax[:, q * QT : (q + 1) * QT],
                    trp.rearrange("p (lt x) -> p lt x", x=P),
                    axis=AX.X,
                )
                # this quarter's w2u = maxP~ * exp(s_in)
                nc.scalar.activation(
                    out=exp_si[:, q * QT : (q + 1) * QT],
                    in_=s_in_sb[:, q * QT : (q + 1) * QT],
                    func=AF.Exp,
                )
                nc.vector.tensor_mul(
                    w2u[:, q * QT : (q + 1) * QT],
                    rowmax[:, q * QT : (q + 1) * QT],
                    exp_si[:, q * QT : (q + 1) * QT],
                )

                # normalize output_one; write blocks 1 and 2 of the output
                for lt in range(QT):
                    tg = q * QT + lt
                    r = rp.tile([P, 1], F32)
                    nc.vector.reciprocal(r[:], accs[lt][:, D : D + 1])
                    nc.vector.tensor_scalar(
                        out=out1_sb[:, tg, :], in0=accs[lt][:, 0:D],
                        scalar1=r[:], scalar2=None, op0=ALU.mult,
                    )
                    st = sp.tile([P, D], F32)
                    nc.gpsimd.tensor_mul(st[:], in_sb[:, tg, :], out1_sb[:, tg, :])
                    nc.sync.dma_start(out=out_r[:, tg, 2 * D : 3 * D], in_=st[:])
                nc.sync.dma_start(
                    out=out_r[:, q * QT : (q + 1) * QT, D : 2 * D],
                    in_=out1_sb[:, q * QT : (q + 1) * QT, :],
                )
                # output block 0 is the input verbatim: straight from SBUF
                nc.sync.dma_start(
                    out=out_r[:, q * QT : (q + 1) * QT, 0:D],
                    in_=in_sb[:, q * QT : (q + 1) * QT, :],
                )

            # ---------------- weight_two tail ----------------
            emit_out2(NQ - 1)
            nc.vector.reduce_sum(w2s[:], w2u[:], axis=AX.X)
            totp = psM.tile([1, 1], F32, tag="m")
            nc.tensor.matmul(totp[:], lhsT=w2s[:], rhs=ones_col[:], start=True, stop=True)
            nc.vector.reciprocal(rtot[:], totp[:])
            # normalized output_two row, broadcast to all partitions (psum)
            o2n = cp.tile([1, D], F32)
            nc.vector.tensor_scalar_mul(o2n[:], in0=o2p[:], scalar1=rtot[:])
            o2bp = psS.tile([P, D], F32, tag="s")
            nc.tensor.matmul(o2bp[:], lhsT=ones_row[:], rhs=o2n[:], start=True, stop=True)
            o2b = cp.tile([P, D], F32)
            nc.scalar.copy(out=o2b[:], in_=o2bp[:])

            for tg in range(LT):
                o4 = sp.tile([P, D], F32, name=f"o4_{tg}", tag="o4", bufs=8)
                if tg % 2 == 0:
                    nc.vector.tensor_mul(o4[:], o2b[:], out1_sb[:, tg, :])
                else:
                    nc.gpsimd.tensor_mul(o4[:], o2b[:], out1_sb[:, tg, :])
                # spread across both HWDGE queues; ACT is idle in the tail
                if tg % 2 == 0:
                    nc.scalar.dma_start(out=out_r[:, tg, 3 * D : 4 * D], in_=o4[:])
                else:
                    nc.sync.dma_start(out=out_r[:, tg, 3 * D : 4 * D], in_=o4[:])

    nc.compile()
    return nc


_CACHE = threading.local()


def _get_nc():
    nc = getattr(_CACHE, "nc", None)
    if nc is None:
        nc = build_nc()
        _CACHE.nc = nc
    return nc


def make_in_maps(input, memory, mask, w_input, w_memory, dot_scale):
    input = np.ascontiguousarray(np.asarray(input, dtype=np.float32))
    memory = np.ascontiguousarray(np.asarray(memory, dtype=np.float32))
    mask = np.ascontiguousarray(np.asarray(mask, dtype=np.float32))
    w_input = np.ascontiguousarray(np.asarray(w_input, dtype=np.float32))
    w_memory = np.ascontiguousarray(np.asarray(w_memory, dtype=np.float32))
    dot_scale = np.ascontiguousarray(np.asarray(dot_scale, dtype=np.float32))
    return [
        {
            "input": input[b],
            "memory": memory[b],
            "mask": mask[b],
            "w_input": w_input,
            "w_memory": w_memory,
            "dot_scale": dot_scale,
        }
        for b in range(B)
    ]


def _run_once(nc, in_maps):
    from concourse.bass_utils import run_bass_kernel_spmd

    res = run_bass_kernel_spmd(nc, in_maps, core_ids=list(range(B)))
    return np.stack([res.results[b]["out"] for b in range(B)], axis=0)


def kernel(input, memory, mask, w_input, w_memory, dot_scale):
    nc = _get_nc()
    in_maps = make_in_maps(input, memory, mask, w_input, w_memory, dot_scale)
    # The kernel is deterministic; rarely a core returns corrupted data after
    # an earlier device fault.  Run twice and require agreement.
    out = _run_once(nc, in_maps)
    for _ in range(3):
        out2 = _run_once(nc, in_maps)
        if np.array_equal(out, out2):
            return out
        out = out2
    return out


# revision 48
# speedup vs baseline: 1.0325x; 1.0325x over previous
"""BiAttention kernel for Trainium2 (Bass/Tile), data-parallel over batch on 8 cores.

Math (per batch b):
  att[l,m] = s_in[l] + g[m] + S[l,m]
    S[l,m]  = sum_d inp[l,d]*dot_scale[d]*mem[m,d]
    s_in[l] = sum_d inp[l,d]*w_input[d]
    g[m]    = sum_d mem[m,d]*w_memory[d] + (mask[m]-1)*1e30
  weight_one = softmax_m(att) = softmax_m(S + g)           (s_in cancels)
  output_one = weight_one @ mem
  w2u[l] = exp(max_m att[l,:]) = max_m exp(S+g) * exp(s_in[l])
  output_two = (w2u/sum w2u) @ inp
  out = concat([inp, output_one, inp*output_one, output_two*output_one], -1)

Implementation: S computed transposed (S_T[m,l]) so that P~ = exp(S_T + g) is
directly the lhsT of the second matmul.  Softmax denominators come for free
from a ones-column appended to mem in the second matmul.  max_m att recovered
from max_m P~ (exp is monotone), so no log is needed:
w2u = maxP~ * exp(s_in).
"""

import threading

import numpy as np

import concourse.bacc as bacc
import concourse.bass as bass
import concourse.mybir as mybir
import concourse.tile as tile
from concourse.masks import make_identity

F32 = mybir.dt.float32
BF16 = mybir.dt.bfloat16
AF = mybir.ActivationFunctionType
ALU = mybir.AluOpType
AX = mybir.AxisListType

B, L, M, D = 8, 2048, 2048, 256
P = 128
LT = L // P          # 16 l-tiles
MT = M // P          # 16 m-tiles
KD = D // P          # 2 contraction tiles
NQ = 4               # l-quarters (PSUM-accumulator constraint)
QW = L // NQ         # 512
QT = QW // P         # 4 l-tiles per quarter
NEG_BIG = 1.0e30


def build_nc():
    nc = bacc.Bacc(
        "TRN2", target_bir_lowering=False, debug=False, num_devices=8
    )

    inp_d = nc.dram_tensor("input", [L, D], F32, kind="ExternalInput").ap()
    mem_d = nc.dram_tensor("memory", [M, D], F32, kind="ExternalInput").ap()
    mask_d = nc.dram_tensor("mask", [M], F32, kind="ExternalInput").ap()
    w_in_d = nc.dram_tensor("w_input", [D], F32, kind="ExternalInput").ap()
    w_mem_d = nc.dram_tensor("w_memory", [D], F32, kind="ExternalInput").ap()
    dsc_d = nc.dram_tensor("dot_scale", [D], F32, kind="ExternalInput").ap()
    out_d = nc.dram_tensor("out", [L, 4 * D], F32, kind="ExternalOutput").ap()

    inp_r = inp_d.rearrange("(t p) d -> p t d", p=P)      # [128,16,256]
    mem_r = mem_d.rearrange("(t p) d -> p t d", p=P)      # [128,16,256]
    mask_r = mask_d.rearrange("(t p) -> t p", p=P)        # [16,128]
    out_r = out_d.rearrange("(t p) c -> p t c", p=P)      # [128,16,1024]

    with tile.TileContext(nc) as tc:
        with (
            tc.tile_pool(name="consts", bufs=1) as cp,
            tc.tile_pool(name="ptiles", bufs=3) as pp,
            tc.tile_pool(name="stage", bufs=4) as sp,
            tc.tile_pool(name="dots", bufs=2) as dp,
            tc.tile_pool(name="rp", bufs=4) as rp,
            tc.tile_pool(name="psS", bufs=2, space="PSUM") as psS,
            tc.tile_pool(name="psM", bufs=2, space="PSUM") as psM,
            tc.tile_pool(name="psAcc", bufs=4, space="PSUM") as psA,
        ):
            # ---------------- persistent SBUF ----------------
            ident_f = cp.tile([P, P], F32)
            ident_b = cp.tile([P, P], BF16)
            make_identity(nc, ident_f)
            make_identity(nc, ident_b)

            in_sb = cp.tile([P, LT, D], F32)       # natural input
            mem_sb = cp.tile([P, MT, D + 1], BF16)  # natural memory + ones col
            inT = cp.tile([P, KD, L], BF16)        # scaled input^T  [d, l]
            memT = cp.tile([P, KD, M], BF16)       # memory^T [d, m]
            maxacc = cp.tile([P, L], BF16)         # running max of P~ over m-tiles
            out1_sb = cp.tile([P, LT, D], F32)     # normalized output_one
            mask_pad = cp.tile([P, P], F32)        # mask rows 0:16, rest garbage
            vpad = cp.tile([P, P], F32)            # dot_scale rows 0:2, w_memory rows 2:4
            w_in_row = cp.tile([1, D], F32)
            w_in_bc = cp.tile([P, D], F32)
            w_mem_sb = cp.tile([P, KD], BF16)
            dsc_sb = cp.tile([P, KD], F32)
            g_sb = cp.tile([P, MT], F32)
            gtmp_sb = cp.tile([P, MT], F32)
            s_in_sb = cp.tile([P, LT], F32)
            exp_si = cp.tile([P, LT], F32)
            rowmax = cp.tile([P, LT], F32)
            w2u = cp.tile([P, LT], F32)
            w2s = cp.tile([P, 1], F32)
            ones_col = cp.tile([P, 1], F32)
            ones_row = cp.tile([1, P], F32)
            rtot = cp.tile([1, 1], F32)
            rtot_b = cp.tile([P, 1], F32)

            # ---------------- loads (small params first) ----------------
            nc.sync.dma_start(out=vpad[0:KD, :], in_=dsc_d.rearrange("(k p) -> k p", p=P))
            nc.sync.dma_start(
                out=vpad[KD : 2 * KD, :], in_=w_mem_d.rearrange("(k p) -> k p", p=P)
            )
            nc.sync.dma_start(out=w_in_row[:], in_=w_in_d[None, :])
            nc.sync.dma_start(out=mask_pad[0:MT, :], in_=mask_r)
            nc.gpsimd.memset(ones_col[:], 1.0)
            nc.gpsimd.memset(ones_row[:], 1.0)
            # touch Exp early so the ACT table load happens off the critical path
            warm = cp.tile([P, 1], F32)
            nc.scalar.activation(out=warm[:], in_=ones_col[:], func=AF.Exp)
            for c in range(8):  # 2-tile chunks matching transpose batches
                nc.sync.dma_start(
                    out=in_sb[:, c * 2 : (c + 1) * 2, :],
                    in_=inp_r[:, c * 2 : (c + 1) * 2, :],
                )
                nc.gpsimd.dma_start(
                    out=mem_sb[:, c * 2 : (c + 1) * 2, 0:D],
                    in_=mem_r[:, c * 2 : (c + 1) * 2, :],
                )  # f32 -> bf16 cast
            nc.gpsimd.memset(mem_sb[:, :, D : D + 1], 1.0)

            # ---------------- small params ----------------
            pv = psS.tile([P, P], F32, tag="s")
            nc.tensor.transpose(pv[:], vpad[:], ident_f[:])
            nc.vector.tensor_copy(dsc_sb[:], pv[:, 0:KD])
            nc.vector.tensor_copy(w_mem_sb[:], pv[:, KD : 2 * KD])  # cast to bf16
            # broadcast w_input across partitions via ones-matmul
            wb = psS.tile([P, D], F32, tag="s")
            nc.tensor.matmul(wb[:], lhsT=ones_row[:], rhs=w_in_row[:], start=True, stop=True)
            nc.vector.tensor_copy(w_in_bc[:], wb[:])

            # mask term: (mask-1)*1e30 via padded full-K transpose
            mtp = psS.tile([P, P], F32, tag="s")  # mask transposed (cols 0:16 valid)
            nc.tensor.transpose(mtp[:], mask_pad[:], ident_f[:])
            nc.vector.tensor_scalar(
                out=gtmp_sb[:], in0=mtp[:, 0:MT], scalar1=1.0, scalar2=NEG_BIG,
                op0=ALU.subtract, op1=ALU.mult,
            )

            # ---------------- big transposes ----------------
            # batch 4 [128,128] transposes per psum tile; batch order (k, t)
            # matches dest AP [p, k, t*128:(t+2)*128].
            # Order: input batches 0-1 (cover all of q=0), then memory batches
            # (gate g and mm1 lhsT), then remaining input batches.
            mdp = psA.tile([P, MT], F32, tag="acc")
            slot = [0]

            def input_batch(bi):
                t0 = 2 * bi
                pool = psA if slot[0] % 2 else psS
                slot[0] += 1
                ptr = pool.tile(
                    [P, 512], F32, name=f"ptri{bi}",
                    tag="acc" if pool is psA else "s",
                )
                j = 0
                for k in range(KD):
                    for t in (t0, t0 + 1):
                        nc.tensor.transpose(
                            ptr[:, j * P : (j + 1) * P],
                            in_sb[:, t, k * P : (k + 1) * P],
                            ident_f,
                        )
                        j += 1
                for k in range(KD):
                    if bi < 2:
                        # ACT is idle before the exp stream starts
                        nc.scalar.activation(
                            out=inT[:, k, t0 * P : (t0 + 2) * P],
                            in_=ptr[:, k * 2 * P : (k + 1) * 2 * P],
                            func=AF.Copy, scale=dsc_sb[:, k : k + 1],
                        )
                    else:
                        # keep ACT's in-order queue free for main-loop exps
                        nc.vector.tensor_scalar(
                            out=inT[:, k, t0 * P : (t0 + 2) * P],
                            in0=ptr[:, k * 2 * P : (k + 1) * 2 * P],
                            scalar1=dsc_sb[:, k : k + 1], scalar2=None,
                            op0=ALU.mult,
                        )

            def memory_batch(bi):
                t0 = 2 * bi
                pool = psA if slot[0] % 2 else psS
                slot[0] += 1
                ptr = pool.tile(
                    [P, 512], BF16, name=f"ptrm{bi}",
                    tag="acc" if pool is psA else "s",
                )
                j = 0
                for k in range(KD):
                    for t in (t0, t0 + 1):
                        nc.tensor.transpose(
                            ptr[:, j * P : (j + 1) * P],
                            mem_sb[:, t, k * P : (k + 1) * P],
                            ident_b,
                        )
                        j += 1
                nc.vector.tensor_copy(
                    out=memT[:, :, t0 * P : (t0 + 2) * P], in_=ptr[:]
                )
                # memory_dot for these two m-tiles, then the g chunk
                for t in (t0, t0 + 1):
                    for k in range(KD):
                        nc.tensor.matmul(
                            mdp[:, t : t + 1],
                            lhsT=memT[:, k, t * P : (t + 1) * P],
                            rhs=w_mem_sb[:, k : k + 1],
                            start=(k == 0),
                            stop=(k == KD - 1),
                        )
                nc.vector.tensor_add(
                    g_sb[:, t0 : t0 + 2],
                    gtmp_sb[:, t0 : t0 + 2],
                    mdp[:, t0 : t0 + 2],
                )

            input_batch(0)
            input_batch(1)
            for bi in range(8):
                memory_batch(bi)
            for bi in range(2, 8):
                input_batch(bi)

            # ---------------- main loop ----------------
            # mm1 runs one iteration ahead of exp/mm2 so the PE never idles
            # waiting on the ACT exp; o2p accumulates output_two per quarter
            # (each quarter's 4 matmuls are emitted mid-next-quarter).
            o2p = psS.tile([1, D], F32, tag="s")

            def emit_mm1(q, t, ps):
                for k in range(KD):
                    nc.tensor.matmul(
                        ps[:],
                        lhsT=memT[:, k, t * P : (t + 1) * P],
                        rhs=inT[:, k, q * QW : (q + 1) * QW],
                        start=(k == 0),
                        stop=(k == KD - 1),
                    )

            def emit_out2(qd):
                for lt in range(QT):
                    tg = qd * QT + lt
                    nc.tensor.matmul(
                        o2p[:],
                        lhsT=w2u[:, tg : tg + 1],
                        rhs=in_sb[:, tg, :],
                        start=(tg == 0),
                        stop=(tg == LT - 1),
                    )

            ps_next = psM.tile([P, QW], F32, tag="m", name="ps_q0_t0")
            emit_mm1(0, 0, ps_next)
            for q in range(NQ):
                accs = [
                    psA.tile([P, D + 1], F32, tag="acc", name=f"acc_q{q}_{i}")
                    for i in range(QT)
                ]
                for t in range(MT):
                    ps = ps_next
                    nt = q * MT + t + 1
                    if nt < NQ * MT:
                        ps_next = psM.tile(
                            [P, QW], F32, tag="m", name=f"ps_{nt}"
                        )
                        emit_mm1(nt // MT, nt % MT, ps_next)
                    pt = pp.tile([P, QW], BF16)
                    nc.scalar.activation(
                        out=pt[:], in_=ps[:], func=AF.Exp, bias=g_sb[:, t : t + 1]
                    )
                    msl = maxacc[:, q * QW : (q + 1) * QW]
                    if t == 0:
                        nc.vector.tensor_copy(msl, pt[:])
                    else:
                        nc.vector.tensor_max(msl, msl, pt[:])
                    # s_in dot-products for this quarter's l-tiles
                    if t % 4 == 1:
                        td = q * QT + t // 4
                        dump = dp.tile([P, D], F32, name=f"dmp_i{td}", tag="dump")
                        nc.vector.tensor_mul(dump[:], in_sb[:, td, :], w_in_bc[:])
                        nc.vector.reduce_sum(
                            s_in_sb[:, td : td + 1], dump[:], axis=AX.X
                        )
                    # previous quarter's output_two matmuls, mid-quarter
                    if t == 6 and q > 0:
                        emit_out2(q - 1)
                    for lt in range(QT):
                        nc.tensor.matmul(
                            accs[lt][:],
                            lhsT=pt[:, lt * P : (lt + 1) * P],
                            rhs=mem_sb[:, t, :],
                            start=(t == 0),
                            stop=(t == MT - 1),
                        )

                # row max over partitions via PE transpose + free-dim reduce
                trp = psS.tile([P, QW], BF16, tag="s")
                for lt in range(QT):
                    nc.tensor.transpose(
                        trp[:, lt * P : (lt + 1) * P],
                        maxacc[:, (q * QT + lt) * P : (q * QT + lt + 1) * P],
                        ident_b,
                    )
                nc.vector.reduce_max(
                    rowmax[:, q * QT : (q + 1) * QT],
                    trp.rearrange("p (lt x) -> p lt x", x=P),
                    axis=AX.X,
                )
                # this quarter's w2u = maxP~ * exp(s_in)
                nc.scalar.activation(
                    out=exp_si[:, q * QT : (q + 1) * QT],
                    in_=s_in_sb[:, q * QT : (q + 1) * QT],
                    func=AF.Exp,
                )
                nc.vector.tensor_mul(
                    w2u[:, q * QT : (q + 1) * QT],
                    rowmax[:, q * QT : (q + 1) * QT],
                    exp_si[:, q * QT : (q + 1) * QT],
                )

                # normalize output_one; write blocks 1 and 2 of the output
                for lt in range(QT):
                    tg = q * QT + lt
                    r = rp.tile([P, 1], F32)
                    nc.vector.reciprocal(r[:], accs[lt][:, D : D + 1])
                    nc.vector.tensor_scalar(
                        out=out1_sb[:, tg, :], in0=accs[lt][:, 0:D],
                        scalar1=r[:], scalar2=None, op0=ALU.mult,
                    )
                    st = sp.tile([P, D], F32)
                    nc.gpsimd.tensor_mul(st[:], in_sb[:, tg, :], out1_sb[:, tg, :])
                    nc.sync.dma_start(out=out_r[:, tg, 2 * D : 3 * D], in_=st[:])
                nc.sync.dma_start(
                    out=out_r[:, q * QT : (q + 1) * QT, D : 2 * D],
                    in_=out1_sb[:, q * QT : (q + 1) * QT, :],
                )
                # output block 0 is the input verbatim: straight from SBUF
                nc.sync.dma_start(
                    out=out_r[:, q * QT : (q + 1) * QT, 0:D],
                    in_=in_sb[:, q * QT : (q + 1) * QT, :],
                )

            # ---------------- weight_two tail ----------------
            emit_out2(NQ - 1)
            nc.vector.reduce_sum(w2s[:], w2u[:], axis=AX.X)
            totp = psM.tile([1, 1], F32, tag="m")
            nc.tensor.matmul(totp[:], lhsT=w2s[:], rhs=ones_col[:], start=True, stop=True)
            nc.vector.reciprocal(rtot[:], totp[:])
            # normalized output_two row, broadcast to all partitions (psum)
            o2n = cp.tile([1, D], F32)
            nc.vector.tensor_scalar_mul(o2n[:], in0=o2p[:], scalar1=rtot[:])
            o2bp = psS.tile([P, D], F32, tag="s")
            nc.tensor.matmul(o2bp[:], lhsT=ones_row[:], rhs=o2n[:], start=True, stop=True)
            o2b = cp.tile([P, D], F32)
            nc.scalar.copy(out=o2b[:], in_=o2bp[:])

            for tg in range(LT):
                o4 = sp.tile([P, D], F32, name=f"o4_{tg}", tag="o4", bufs=8)
                if tg % 2 == 0:
                    nc.vector.tensor_mul(o4[:], o2b[:], out1_sb[:, tg, :])
                else:
                    nc.gpsimd.tensor_mul(o4[:], o2b[:], out1_sb[:, tg, :])
                # spread across both HWDGE queues; ACT is idle in the tail
                if tg % 2 == 0:
                    nc.scalar.dma_start(out=out_r[:, tg, 3 * D : 4 * D], in_=o4[:])
                else:
                    nc.sync.dma_start(out=out_r[:, tg, 3 * D : 4 * D], in_=o4[:])

    nc.compile()
    return nc


_CACHE = threading.local()


def _get_nc():
    nc = getattr(_CACHE, "nc", None)
    if nc is None:
        nc = build_nc()
        _CACHE.nc = nc
    return nc


def make_in_maps(input, memory, mask, w_input, w_memory, dot_scale):
    input = np.ascontiguousarray(np.asarray(input, dtype=np.float32))
    memory = np.ascontiguousarray(np.asarray(memory, dtype=np.float32))
    mask = np.ascontiguousarray(np.asarray(mask, dtype=np.float32))
    w_input = np.ascontiguousarray(np.asarray(w_input, dtype=np.float32))
    w_memory = np.ascontiguousarray(np.asarray(w_memory, dtype=np.float32))
    dot_scale = np.ascontiguousarray(np.asarray(dot_scale, dtype=np.float32))
    return [
        {
            "input": input[b],
            "memory": memory[b],
            "mask": mask[b],
            "w_input": w_input,
            "w_memory": w_memory,
            "dot_scale": dot_scale,
        }
        for b in range(B)
    ]


def _run_once(nc, in_maps):
    from concourse.bass_utils import run_bass_kernel_spmd

    res = run_bass_kernel_spmd(nc, in_maps, core_ids=list(range(B)))
    return np.stack([res.results[b]["out"] for b in range(B)], axis=0)


def kernel(input, memory, mask, w_input, w_memory, dot_scale):
    nc = _get_nc()
    in_maps = make_in_maps(input, memory, mask, w_input, w_memory, dot_scale)
    # The kernel is deterministic; rarely a core returns corrupted data after
    # an earlier device fault.  Run twice and require agreement.
    out = _run_once(nc, in_maps)
    for _ in range(3):
        out2 = _run_once(nc, in_maps)
        if np.array_equal(out, out2):
            return out
        out = out2
    return out
